# revision 15
# baseline (speedup 1.0000x reference)
"""MultiHeadLatentAttention Trainium2 kernel (8 NeuronCores), v2.

Sharding: core c -> (batch b = c//2, head-group hg = c%2, 8 heads each),
heads in 2 groups g of 4.

Key structure (vs v1): one global software pipeline so the ACT engine
(softmax exp, the roofline bottleneck at ~150us/core) never starves:
  - Latent projections (fused q2l/kv2l folded into c_attn on host) are
    emitted as sub-units dispensed between attention iterations.
  - Attention in [keys, queries] layout, 256-query blocks, 4 heads per
    exp instruction; scores row-group-tiled (4 concurrent K=32 matmuls),
    att@V col-group-tiled (4 concurrent M=32 matmuls).
  - Softmax denominator via an all-ones [128,32] stationary operand:
    z comes out replicated over the 32 latent partitions of each head, so
    normalization is reciprocal + one multiply straight out of PSUM.
  - 2-rank AllGather (pairs) of normalized latents, chunked per 512
    queries and overlapped; folded l2o+cproj ("W2") output projection
    per chunk, overlapped with later attention blocks.
"""
import sys

if "/opt/trn_rl_repo" not in sys.path:
    sys.path.insert(0, "/opt/trn_rl_repo")

import numpy as np
import ml_dtypes

import concourse.bass as bass
import concourse.tile as tile
from concourse.tile_rust import add_dep_helper
from concourse import bacc, mybir
from concourse.bass_utils import run_bass_kernel_spmd

F32 = mybir.dt.float32
F32R = mybir.dt.float32r
BF16 = mybir.dt.bfloat16

B, T, C = 4, 2048, 1024
H, HD, L = 16, 64, 32
NCORES = 8
QB = 256               # query block
NQB = T // QB          # 8
NB = T // 512          # 4 projection column blocks
KC = T // 128          # 16 key chunks of 128
SCALE = float(1.0 / np.sqrt(L))
REPLICA_GROUPS = [[0, 1], [2, 3], [4, 5], [6, 7]]

_CACHE = {}


def build_program(repeat=1):
    nc = bacc.Bacc("TRN2", target_bir_lowering=False, debug=False,
                   num_devices=NCORES)
    xT = nc.dram_tensor("xT", [C, T], F32R, kind="ExternalInput").ap()
    wlat = nc.dram_tensor("wlat", [C, 768], F32R, kind="ExternalInput").ap()
    qkbias = nc.dram_tensor("qkbias", [128, 4], F32, kind="ExternalInput").ap()
    p4 = nc.dram_tensor("p4", [128, 128], BF16, kind="ExternalInput").ap()
    ropec = nc.dram_tensor("ropec", [128, T], BF16, kind="ExternalInput").ap()
    ropes = nc.dram_tensor("ropes", [128, T], BF16, kind="ExternalInput").ap()
    wqk4 = nc.dram_tensor("wqk4", [128, 32], BF16, kind="ExternalInput").ap()
    wqkb4 = nc.dram_tensor("wqkb4", [128, 1], F32, kind="ExternalInput").ap()
    mask4 = nc.dram_tensor("mask4", [128, 512], BF16, kind="ExternalInput").ap()
    w2 = nc.dram_tensor("w2", [512, 512], BF16, kind="ExternalInput").ap()
    bout = nc.dram_tensor("bout", [128, 4], F32, kind="ExternalInput").ap()
    outT = nc.dram_tensor("outT", [512, T], F32, kind="ExternalOutput").ap()

    with tile.TileContext(nc) as tc:
        for _rep in range(repeat):
            _emit_body(nc, tc, xT, wlat, qkbias, p4, ropec, ropes, wqk4,
                       wqkb4, mask4, w2, bout, outT)
    nc.compile()
    return nc


def _emit_body(nc, tc, xT, wlat, qkbias, p4, ropec, ropes, wqk4, wqkb4,
               mask4, w2, bout, outT):
    Iden = mybir.ActivationFunctionType.Identity
    Exp = mybir.ActivationFunctionType.Exp

    with tc.tile_pool(name="persist", bufs=1) as pp:
        wqk_t = pp.tile([128, 32], BF16, name="wqk_t")
        nc.sync.dma_start(wqk_t[:], wqk4[:])
        wqkb_t = pp.tile([128, 1], F32, name="wqkb_t")
        nc.sync.dma_start(wqkb_t[:], wqkb4[:])
        qkb_t = pp.tile([128, 4], F32, name="qkb_t")
        nc.sync.dma_start(qkb_t[:], qkbias[:])
        bout_t = pp.tile([128, 4], F32, name="bout_t")
        nc.sync.dma_start(bout_t[:], bout[:])
        mask_t = pp.tile([128, 512], BF16, name="mask_t")
        nc.sync.dma_start(mask_t[:], mask4[:])
        p4_t = pp.tile([128, 128], BF16, name="p4_t")
        nc.sync.dma_start(p4_t[:], p4[:])
        w2t = [pp.tile([128, 512], BF16, name=f"w2t{j}") for j in range(4)]
        for j in range(4):
            nc.sync.dma_start(w2t[j][:], w2[128 * j:128 * (j + 1), :])
        cos_t = pp.tile([128, T], BF16, name="cos_t")
        nc.sync.dma_start(cos_t[:], ropec[:])
        sin_t = pp.tile([128, T], BF16, name="sin_t")
        nc.sync.dma_start(sin_t[:], ropes[:])
        wl = pp.tile([128, 8 * 768], F32R, name="wl")
        for k in range(8):
            nc.sync.dma_start(wl[:, k * 768:(k + 1) * 768],
                              wlat[k * 128:(k + 1) * 128, :])
        ones_t = pp.tile([128, 32], BF16, name="ones_t")
        nc.vector.memset(ones_t[:], 1.0)

        # persistent activations; heads h at partitions 32*(h%2)..+32,
        # column half h//2 (so score waves serialize on array rows instead
        # of clashing on psum banks)
        QF = [pp.tile([64, 2 * T], BF16, name=f"QF{g}") for g in range(2)]
        KR = [pp.tile([64, 2 * T], BF16, name=f"KR{g}") for g in range(2)]
        VA = [pp.tile([128, KC * 128], BF16, name=f"VA{g}") for g in range(2)]

        with tc.tile_pool(name="work", bufs=1) as pw, \
             tc.tile_pool(name="ps", bufs=1, space="PSUM") as psp, \
             tc.tile_pool(name="dram", bufs=1, space="DRAM") as dr:
            ybounce = [dr.tile([256, 512], BF16, name=f"ybounce{p}")
                       for p in range(NB)]
            ygath = [dr.tile([512, 512], BF16, name=f"ygath{p}")
                     for p in range(NB)]
            ybounce_h = [dr.tile([256, 256], BF16, name=f"ybounce_h{i}")
                         for i in range(2)]
            ygath_h = [dr.tile([512, 256], BF16, name=f"ygath_h{i}")
                       for i in range(2)]

            xts = {}

            def load_xts(nb4):
                t = pw.tile([128, 8 * 512], F32R, name=f"xts{nb4}",
                            tag="xts", bufs=2)
                for k in range(8):
                    nc.sync.dma_start(
                        t[:, k * 512:(k + 1) * 512],
                        xT[k * 128:(k + 1) * 128, nb4 * 512:(nb4 + 1) * 512])
                xts[nb4] = t

            # ---------------- phase-A sub-unit emitters ---------------------
            def unit_lat(nb4, g, kind, evict_act):
                """x->latent projection + rope for 4 heads (one m-chunk)."""
                sl = slice(nb4 * 512, (nb4 + 1) * 512)
                mcol = (0 if kind == "q" else 2) + g
                lat_ps = psp.tile([128, 512], F32, name="lat_ps", tag="lat",
                                  bufs=2)
                for k in range(8):
                    nc.tensor.matmul(
                        lat_ps[:],
                        wl[:, k * 768 + mcol * 128: k * 768 + (mcol + 1) * 128],
                        xts[nb4][:, k * 512:(k + 1) * 512],
                        start=(k == 0), stop=(k == 7))
                a_sb = pw.tile([128, 512], BF16, name="a_sb", tag="a_sb",
                               bufs=3)
                if evict_act:
                    nc.scalar.activation(a_sb[:], lat_ps[:], Iden,
                                         bias=qkb_t[:, mcol:mcol + 1])
                else:
                    nc.vector.tensor_scalar_add(a_sb[:], lat_ps[:],
                                                qkb_t[:, mcol:mcol + 1])
                rot_ps = psp.tile([128, 512], F32, name="rot_ps", tag="lat",
                                  bufs=2)
                nc.tensor.matmul(rot_ps[:], p4_t[:], a_sb[:],
                                 start=True, stop=True)
                eng = nc.vector if _CACHE.get("no_gpsimd") else nc.gpsimd
                t1 = pw.tile([128, 512], BF16, name="t1", tag="t1", bufs=2)
                eng.tensor_mul(t1[:], a_sb[:], cos_t[:, sl])
                t2 = pw.tile([128, 512], BF16, name="t2", tag="t2", bufs=2)
                nc.vector.tensor_mul(t2[:], rot_ps[:], sin_t[:, sl])
                if kind == "k":
                    eng.tensor_add(KR[g][:, sl], t1[0:64, :], t2[0:64, :])
                    eng.tensor_add(KR[g][:, T + nb4 * 512:T + (nb4 + 1) * 512],
                                   t1[64:128, :], t2[64:128, :])
                else:
                    qro = pw.tile([128, 512], BF16, name="qro", tag=f"qro{g}",
                                  bufs=2)
                    eng.tensor_add(qro[:], t1[:], t2[:])
                    return qro

            def unit_wqk(nb4, g, qro, evict_act):
                sl = slice(nb4 * 512, (nb4 + 1) * 512)
                wq_ps = psp.tile([128, 512], F32, name="wq_ps", tag="lat",
                                 bufs=2)
                for h in range(4):
                    nc.tensor.matmul(wq_ps[32 * h:32 * h + 32, :],
                                     wqk_t[32 * h:32 * h + 32, :],
                                     qro[32 * h:32 * h + 32, :],
                                     start=True, stop=True,
                                     tile_position=(32 * h, 32 * h))
                sl2 = slice(T + nb4 * 512, T + (nb4 + 1) * 512)
                if evict_act:
                    nc.scalar.activation(QF[g][:, sl], wq_ps[0:64, :], Iden,
                                         bias=wqkb_t[0:64, 0:1])
                    nc.scalar.activation(QF[g][:, sl2], wq_ps[64:128, :], Iden,
                                         bias=wqkb_t[0:64, 0:1])
                else:
                    nc.vector.tensor_scalar_add(QF[g][:, sl], wq_ps[0:64, :],
                                                wqkb_t[0:64, 0:1])
                    nc.vector.tensor_scalar_add(QF[g][:, sl2], wq_ps[64:128, :],
                                                wqkb_t[0:64, 0:1])

            def unit_v(nb4, tck2):
                """V latents for token chunk tck = 4*nb4 + tck2 (both g)."""
                tck = 4 * nb4 + tck2
                v_ps = psp.tile([128, 512], F32, name="v_ps", tag="lat",
                                bufs=2)
                for k in range(8):
                    nc.tensor.matmul(
                        v_ps[:, 0:256],
                        xts[nb4][:, k * 512 + tck2 * 128: k * 512 + tck2 * 128 + 128],
                        wl[:, k * 768 + 512: k * 768 + 768],
                        start=(k == 0), stop=(k == 7))
                for g in range(2):
                    nc.vector.tensor_copy(
                        VA[g][:, tck * 128:(tck + 1) * 128],
                        v_ps[:, 128 * g:128 * (g + 1)])

            def emit_a_units(nb4, evict_act, gs=(0, 1), with_v=True):
                units = []
                for g in gs:
                    units.append(lambda g=g: unit_lat(nb4, g, "k", evict_act))

                    def qchain(g=g):
                        qro = unit_lat(nb4, g, "q", evict_act)
                        unit_wqk(nb4, g, qro, evict_act)
                    units.append(qchain)
                if with_v:
                    for tck2 in range(4):
                        units.append(lambda t=tck2: unit_v(nb4, t))
                return units

            # ---------------- phase-B unit -----------------------------------
            def emit_b(g, qb2, dispense):
                q0 = qb2 * QB
                nch = 2 * qb2 + 2
                so = _CACHE.get("scores_only")
                yz = None if so else psp.tile([128, 512], F32, name="yz",
                                              tag="acc", bufs=2)
                for kc in range(nch):
                    k0 = kc * 128
                    c0 = max(0, k0 - q0)
                    ncol = QB - c0
                    sp = psp.tile([128, 1024], F32, name="sp", tag="sp",
                                  bufs=2)
                    # region r = 2*(h%2) + h//2: h0->0, h1->2, h2->1, h3->3.
                    # waves (h0,h1) then (h2,h3): same array rows as wave 0,
                    # so they serialize on the PE instead of clashing on the
                    # bank; their regions share banks with wave 0, hence
                    # start=False + pending-zero from wave 0's start.
                    smm = {}
                    for h in ([] if _CACHE.get("no_scores") else range(4)):
                        row = 32 * (h % 2)
                        half = h // 2
                        r = 2 * (h % 2) + half
                        first = (half == 0)
                        smm[h] = nc.tensor.matmul(
                            sp[:, QB * r + c0: QB * (r + 1)],
                            KR[g][row:row + 32, half * T + k0:half * T + k0 + 128],
                            QF[g][row:row + 32,
                                  half * T + q0 + c0:half * T + q0 + QB],
                            start=first, stop=first,
                            skip_group_check=not first,
                            tile_position=(row, 0))
                        if not first:
                            add_dep_helper(smm[h].ins, smm[h - 2].ins,
                                           sync=False,
                                           reason="psum bank zero-mark order")
                    es = pw.tile([128, 1024], BF16, name="es", tag="es",
                                 bufs=3)
                    esv = es.rearrange("p (h n) -> p h n", h=4)
                    spv = sp.rearrange("p (h n) -> p h n", h=4)
                    if not _CACHE.get("no_scores") and not _CACHE.get("no_exp"):
                        nc.scalar.activation(esv[:, :, c0:QB], spv[:, :, c0:QB],
                                             Exp, scale=SCALE)
                    elif _CACHE.get("no_exp"):
                        nc.vector.tensor_copy(esv[:, :, c0:QB], spv[:, :, c0:QB])
                    if k0 >= q0 and not _CACHE.get("no_scores") and not _CACHE.get("no_mask"):  # diagonal chunk: mask triangle
                        mv = mask_t.rearrange("p (h n) -> p h n", h=4)
                        nc.vector.tensor_mul(esv[:, :, c0:c0 + 128],
                                             esv[:, :, c0:c0 + 128], mv)
                    # y (cols 0:256) and z (cols 256:512) share each bank;
                    # one accumulation group per 32-partition range: opened by
                    # the first y matmul (zero-region mark covers z too),
                    # closed by the last z matmul.
                    ymm = {}
                    for h in ([] if _CACHE.get("no_vz") or so else range(4)):
                        r = 2 * (h % 2) + h // 2
                        ymm[h] = nc.tensor.matmul(
                            yz[32 * h:32 * h + 32, c0:QB],
                            VA[g][:, kc * 128 + 32 * h: kc * 128 + 32 * h + 32],
                            es[:, QB * r + c0:QB * (r + 1)],
                            start=(kc == 0), stop=False,
                            skip_group_check=True,
                            tile_position=(0, 32 * h))
                    for h in ([] if _CACHE.get("no_vz") or _CACHE.get("no_z") or so else range(4)):
                        r = 2 * (h % 2) + h // 2
                        zmm = nc.tensor.matmul(
                            yz[32 * h:32 * h + 32, 256 + c0:256 + QB],
                            ones_t[:],
                            es[:, QB * r + c0:QB * (r + 1)],
                            start=False, stop=(kc == nch - 1),
                            skip_group_check=True,
                            tile_position=(0, 32 * h))
                        if kc == 0:
                            add_dep_helper(zmm.ins, ymm[h].ins, sync=False,
                                           reason="psum bank zero-mark order")
                    dispense()
                if so:
                    return
                # normalize straight out of PSUM (z replicated per latent row)
                r32 = pw.tile([128, 256], F32, name="r32", tag="r32", bufs=2)
                nc.vector.reciprocal(r32[:], yz[:, 256:512])
                yn = pw.tile([128, 256], BF16, name="yn", tag="yn", bufs=2)
                nc.vector.tensor_mul(yn[:], yz[:, 0:256], r32[:])
                p = qb2 // 2
                if p == NB - 1 and _CACHE.get("tail_split", 1):
                    nc.sync.dma_start(
                        ybounce_h[qb2 % 2][128 * g:128 * (g + 1), :], yn[:])
                else:
                    nc.sync.dma_start(
                        ybounce[p][128 * g:128 * (g + 1),
                                   256 * (qb2 % 2):256 * (qb2 % 2) + 256],
                        yn[:])

            # ---------------- W2 output projection ---------------------------
            def emit_w2(p):
                ygr = []
                for j in range(4):
                    t = pw.tile([128, 512], BF16, name=f"ygr{p}_{j}",
                                tag="ygr", bufs=8)
                    nc.sync.dma_start(t[:], ygath[p][128 * j:128 * (j + 1), :])
                    ygr.append(t)
                for m in range(4):
                    ps_o = psp.tile([128, 512], F32, name="ps_o", tag="acc",
                                    bufs=2)
                    for j in range(4):
                        nc.tensor.matmul(ps_o[:],
                                         w2t[j][:, m * 128:(m + 1) * 128],
                                         ygr[j][:],
                                         start=(j == 0), stop=(j == 3))
                    o_sb = pw.tile([128, 512], F32, name="o_sb", tag="o_sb",
                                   bufs=2)
                    nc.vector.tensor_scalar_add(o_sb[:], ps_o[:],
                                                bout_t[:, m:m + 1])
                    nc.sync.dma_start(outT[m * 128:(m + 1) * 128,
                                           512 * p:512 * (p + 1)], o_sb[:])

            def emit_gather_h(i):
                if _CACHE.get("no_collective"):
                    nc.sync.dma_start(ygath_h[i][0:256, :], ybounce_h[i][:])
                    nc.sync.dma_start(ygath_h[i][256:512, :], ybounce_h[i][:])
                else:
                    nc.gpsimd.collective_compute(
                        "AllGather", mybir.AluOpType.bypass,
                        replica_groups=REPLICA_GROUPS,
                        ins=[ybounce_h[i].opt()], outs=[ygath_h[i].opt()])

            def emit_w2_h(i):
                qb2 = 2 * (NB - 1) + i
                ygr = []
                for j in range(4):
                    t = pw.tile([128, 256], BF16, name=f"ygrh{i}_{j}",
                                tag="ygrh", bufs=8)
                    nc.sync.dma_start(t[:], ygath_h[i][128 * j:128 * (j + 1), :])
                    ygr.append(t)
                for m in range(4):
                    ps_o = psp.tile([128, 512], F32, name="ps_o", tag="acc",
                                    bufs=2)
                    for j in range(4):
                        nc.tensor.matmul(ps_o[:, 0:256],
                                         w2t[j][:, m * 128:(m + 1) * 128],
                                         ygr[j][:],
                                         start=(j == 0), stop=(j == 3))
                    o_sb = pw.tile([128, 256], F32, name="o_sbh", tag="o_sbh",
                                   bufs=2)
                    nc.vector.tensor_scalar_add(o_sb[:], ps_o[:, 0:256],
                                                bout_t[:, m:m + 1])
                    nc.sync.dma_start(outT[m * 128:(m + 1) * 128,
                                           256 * qb2:256 * (qb2 + 1)], o_sb[:])

            def emit_gather(p):
                if _CACHE.get("no_collective"):
                    nc.sync.dma_start(ygath[p][0:256, :], ybounce[p][:])
                    nc.sync.dma_start(ygath[p][256:512, :], ybounce[p][:])
                else:
                    nc.gpsimd.collective_compute(
                        "AllGather", mybir.AluOpType.bypass,
                        replica_groups=REPLICA_GROUPS,
                        ins=[ybounce[p].opt()], outs=[ygath[p].opt()])

            # ---------------- top-level schedule -----------------------------
            load_xts(0)
            # head of A(0): everything B(g0, qb 0/1) needs, evictions on ACT
            for u in emit_a_units(0, evict_act=True, gs=(0,), with_v=False):
                u()
            for tck2 in range(4):
                unit_v(0, tck2)

            if _CACHE.get("a_only"):
                for u in emit_a_units(0, evict_act=True, gs=(1,), with_v=False):
                    u()
                for nb4 in range(1, 4):
                    load_xts(nb4)
                    for u in emit_a_units(nb4, evict_act=True):
                        u()
                return
            # dispensers per block: remaining A units, paced over B iters
            pending = {
                0: emit_a_units(0, evict_act=False, gs=(1,), with_v=False)
                + emit_a_units(1, evict_act=False),
                1: emit_a_units(2, evict_act=False),
                2: emit_a_units(3, evict_act=False),
                3: [],
            }
            if _CACHE.get("no_dispense"):
                for pbl in range(4):
                    if pbl < 3:
                        load_xts(pbl + 1)
                    for u in pending[pbl]:
                        u()
                    pending[pbl] = []
            iters_in_block = {p: 2 * ((4 * p + 2) + (4 * p + 4))
                              for p in range(NB)}

            for p in range(NB):
                if p < 3:
                    load_xts(p + 1)
                units = pending[p]
                n_iter = iters_in_block[p]
                state = {"i": 0, "done": 0}

                def dispense(units=units, n_iter=n_iter, state=state):
                    state["i"] += 1
                    target = (state["i"] * len(units) + n_iter - 1) // n_iter
                    while state["done"] < min(target, len(units)):
                        units[state["done"]]()
                        state["done"] += 1

                last = (p == NB - 1) and _CACHE.get("tail_split", 1) \
                    and not _CACHE.get("no_w2")
                if last:
                    unit_order = [(0, 2 * p), (1, 2 * p),
                                  (0, 2 * p + 1), (1, 2 * p + 1)]
                else:
                    unit_order = [(0, 2 * p), (0, 2 * p + 1),
                                  (1, 2 * p), (1, 2 * p + 1)]
                bcount = 0
                for g, qb2 in unit_order:
                    if bcount == 2 and p >= 1 and not _CACHE.get("no_w2"):
                        emit_w2(p - 1)
                    emit_b(g, qb2, dispense)
                    bcount += 1
                    if last and bcount == 2:
                        emit_gather_h(0)
                    if last and bcount == 3:
                        emit_w2_h(0)
                while state["done"] < len(units):
                    units[state["done"]]()
                    state["done"] += 1
                if not _CACHE.get("no_w2"):
                    if last:
                        emit_gather_h(1)
                        emit_w2_h(1)
                    else:
                        emit_gather(p)
            if _CACHE.get("no_w2"):
                pass
            elif not _CACHE.get("tail_split", 1):
                emit_w2(NB - 1)


# ---------------------------------------------------------------------------
# Host-side input preparation
# ---------------------------------------------------------------------------

def prepare_inputs(inputs):
    """Fold weights and build the 8 per-core input maps."""
    x = np.ascontiguousarray(np.asarray(inputs["x"], dtype=np.float32))
    caw = np.asarray(inputs["c_attn_w"], dtype=np.float32)
    cab = np.asarray(inputs["c_attn_b"], dtype=np.float32)
    q2l = np.asarray(inputs["q2l_w"], dtype=np.float32)
    q2lb = np.asarray(inputs["q2l_b"], dtype=np.float32)
    kv2l = np.asarray(inputs["kv2l_w"], dtype=np.float32)
    kv2lb = np.asarray(inputs["kv2l_b"], dtype=np.float32)
    l2o = np.asarray(inputs["l2o_w"], dtype=np.float32)
    l2ob = np.asarray(inputs["l2o_b"], dtype=np.float32)
    wqk = np.asarray(inputs["wqk_w"], dtype=np.float32)
    wqkb = np.asarray(inputs["wqk_b"], dtype=np.float32)
    cpw = np.asarray(inputs["cproj_w"], dtype=np.float32)
    cpb = np.asarray(inputs["cproj_b"], dtype=np.float32)

    # rope tables [L, T] -> tiled to [128, T]
    inv_freq = 1.0 / (10000.0 ** (np.arange(0, L, 2, dtype=np.float32) / L))
    t_ar = np.arange(T, dtype=np.float32)
    freqs = np.outer(t_ar, inv_freq)
    cosT = np.repeat(np.cos(freqs), 2, axis=-1)[:, :L].T.astype(np.float32)
    sinT = np.repeat(np.sin(freqs), 2, axis=-1)[:, :L].T.astype(np.float32)
    ropec = np.tile(cosT, (4, 1)).astype(ml_dtypes.bfloat16)   # [128, T]
    ropes = np.tile(sinT, (4, 1)).astype(ml_dtypes.bfloat16)

    P = np.zeros((L, L), np.float32)
    for i in range(L // 2):
        P[2 * i, 2 * i + 1] = -1.0
        P[2 * i + 1, 2 * i] = 1.0

    def fold_head(h):
        Wq = caw[h * HD:(h + 1) * HD, :]
        Wk = caw[C + h * HD: C + (h + 1) * HD, :]
        Wv = caw[2 * C + h * HD: 2 * C + (h + 1) * HD, :]
        bq = cab[h * HD:(h + 1) * HD]
        bk = cab[C + h * HD: C + (h + 1) * HD]
        bv = cab[2 * C + h * HD: 2 * C + (h + 1) * HD]
        return (q2l @ Wq, kv2l @ Wk, kv2l @ Wv,
                q2l @ bq + q2lb, kv2l @ bk + kv2lb, kv2l @ bv + kv2lb)

    # W2 [H*L, C] + folded output bias
    W2 = np.zeros((H * L, C), np.float32)
    b_out = cpb.astype(np.float64).copy()
    for h in range(H):
        W2_h = l2o.T @ cpw[:, h * HD:(h + 1) * HD].T
        W2[h * L:(h + 1) * L] = W2_h
        _, _, _, _, _, bvl = fold_head(h)
        b_out += bvl @ W2_h
        b_out += l2ob @ cpw[:, h * HD:(h + 1) * HD].T
    b_out = b_out.astype(np.float32)

    # per-head-group folded projection stacks
    wlat_hg, qkb_hg, bout_hg = [], [], []
    for hg in range(2):
        wlat = np.zeros((C, 768), np.float32)
        qkb = np.zeros((128, 4), np.float32)
        for g in range(2):
            for lh4 in range(4):
                lh = 4 * g + lh4
                h = hg * 8 + lh
                Wql, Wkl, Wvl, bql, bkl, bvl = fold_head(h)
                wlat[:, (0 + g) * 128 + lh4 * 32:(0 + g) * 128 + lh4 * 32 + 32] = Wql.T
                wlat[:, (2 + g) * 128 + lh4 * 32:(2 + g) * 128 + lh4 * 32 + 32] = Wkl.T
                qkb[lh4 * 32:lh4 * 32 + 32, 0 + g] = bql
                qkb[lh4 * 32:lh4 * 32 + 32, 2 + g] = bkl
        for lh in range(8):
            h = hg * 8 + lh
            _, _, Wvl, _, _, _ = fold_head(h)
            wlat[:, 512 + lh * 32: 512 + (lh + 1) * 32] = Wvl.T
        wlat_hg.append(wlat)
        qkb_hg.append(qkb)
        bo = b_out[hg * 512:(hg + 1) * 512]
        bout_hg.append(np.ascontiguousarray(bo.reshape(4, 128).T))

    # w2 per hg: gathered row r = 256*rank + 128*g + 32*h + l == W2 row
    w2_hg = [np.ascontiguousarray(W2[:, hg * 512:(hg + 1) * 512])
             .astype(ml_dtypes.bfloat16) for hg in range(2)]

    wqk4 = np.tile(wqk.T, (4, 1)).astype(ml_dtypes.bfloat16)     # [128, 32]
    wqkb4 = np.tile(wqkb, 4).reshape(128, 1).astype(np.float32)

    i_idx = np.arange(128)[:, None]
    u_idx = np.arange(128)[None, :]
    tri = (u_idx >= i_idx).astype(ml_dtypes.bfloat16)            # [128, 128]
    mask4 = np.tile(tri, (1, 4))                                 # [128, 512]

    # block-diag rotation lhsT: out = p4.T @ lat = P @ lat per 32-block
    p4 = np.zeros((128, 128), np.float32)
    for h in range(4):
        p4[h * 32:(h + 1) * 32, h * 32:(h + 1) * 32] = P.T
    p4 = p4.astype(ml_dtypes.bfloat16)

    xT_b = [np.ascontiguousarray(x[b].T) for b in range(B)]

    in_maps = []
    for core in range(NCORES):
        b, hg = core // 2, core % 2
        in_maps.append({
            "xT": xT_b[b],
            "wlat": wlat_hg[hg],
            "qkbias": qkb_hg[hg],
            "ropec": ropec,
            "ropes": ropes,
            "wqk4": wqk4,
            "wqkb4": wqkb4,
            "mask4": mask4,
            "p4": p4,
            "w2": w2_hg[hg],
            "bout": bout_hg[hg],
        })
    return in_maps


def assemble_output(results):
    out = np.zeros((B, T, C), np.float32)
    for core in range(NCORES):
        b, hg = core // 2, core % 2
        out[b, :, hg * 512:(hg + 1) * 512] = results[core]["outT"].T
    return out


def kernel(**inputs):
    if "nc" not in _CACHE:
        _CACHE["nc"] = build_program()
    nc = _CACHE["nc"]
    in_maps = prepare_inputs(inputs)
    # The neuron runtime is occasionally left unrecoverable by a previous
    # process (NRT_EXEC_UNIT_UNRECOVERABLE); a short wait + retry clears it.
    last = None
    for attempt in range(3):
        try:
            res = run_bass_kernel_spmd(nc, in_maps,
                                       core_ids=list(range(NCORES)))
            return assemble_output(res.results)
        except Exception as e:  # noqa: BLE001
            last = e
            import time as _time
            _time.sleep(10 * (attempt + 1))
    raise last


# ---------------------------------------------------------------------------
# Timing runner (dev/test only): keeps the compiled executable and
# device-staged inputs so repeated executions measure device time + dispatch,
# not host transfers or recompiles.
# ---------------------------------------------------------------------------

class Runner:
    def __init__(self, nc, in_maps):
        import jax
        from jax.sharding import Mesh, PartitionSpec, NamedSharding
        from jax.experimental.shard_map import shard_map
        from concourse import bass2jax, mybir as _mybir

        bass2jax.install_neuronx_cc_hook()
        partition_name = (nc.partition_id_tensor.name
                          if nc.partition_id_tensor else None)
        in_names, out_names, out_avals, zero_outs = [], [], [], []
        for alloc in nc.m.functions[0].allocations:
            if not isinstance(alloc, _mybir.MemoryLocationSet):
                continue
            name = alloc.memorylocations[0].name
            if alloc.kind == "ExternalInput":
                if name != partition_name:
                    in_names.append(name)
            elif alloc.kind == "ExternalOutput":
                shape = tuple(alloc.tensor_shape)
                dtype = _mybir.dt.np(alloc.dtype)
                out_names.append(name)
                out_avals.append(jax.core.ShapedArray(shape, dtype))
                zero_outs.append(np.zeros(shape, dtype))
        n_params = len(in_names)
        all_names = list(in_names) + list(out_names)
        if partition_name is not None:
            all_names.append(partition_name)
        self.out_names = out_names

        def _body(*args):
            operands = list(args)
            if partition_name is not None:
                operands.append(bass2jax.partition_id_tensor())
            outs = bass2jax._bass_exec_p.bind(
                *operands,
                out_avals=tuple(out_avals),
                in_names=tuple(all_names),
                out_names=tuple(out_names),
                lowering_input_output_aliases=(),
                sim_require_finite=True,
                sim_require_nnan=True,
                nc=nc,
            )
            return tuple(outs)

        devices = jax.devices()[:NCORES]
        mesh = Mesh(np.asarray(devices), ("core",))
        n_out = len(out_names)
        self._fn = jax.jit(shard_map(
            _body, mesh=mesh,
            in_specs=(PartitionSpec("core"),) * (n_params + n_out),
            out_specs=(PartitionSpec("core"),) * n_out,
            check_rep=False))
        sh = NamedSharding(mesh, PartitionSpec("core"))
        concat_in = [
            np.concatenate([np.asarray(in_maps[c][nm]) for c in range(NCORES)],
                           axis=0)
            for nm in in_names]
        concat_zeros = [np.zeros((NCORES * z.shape[0], *z.shape[1:]), z.dtype)
                        for z in zero_outs]
        self._staged = [jax.device_put(a, sh) for a in concat_in + concat_zeros]
        self._out_shapes = [a.shape for a in zero_outs]

    def run(self):
        import jax
        outs = self._fn(*self._staged)
        jax.block_until_ready(outs)
        return outs

    def results(self):
        outs = self.run()
        res = []
        for c in range(NCORES):
            d = {}
            for i, nm in enumerate(self.out_names):
                s0 = self._out_shapes[i][0]
                d[nm] = np.asarray(outs[i]).reshape(NCORES, s0, -1)[c]
            res.append(d)
        return res


if __name__ == "__main__":
    data = dict(np.load("/root/problem/inputs.npz"))
    expected = np.load("/root/problem/expected.npy")
    got = kernel(**data)
    err = np.abs(got - expected)
    print(f"absmax={err.max():.3e} rel={err.max() / np.abs(expected).max():.3e}")


# revision 18
# speedup vs baseline: 1.3167x; 1.3167x over previous
"""MultiHeadLatentAttention Trainium2 kernel (8 NeuronCores), v2.

Sharding: core c -> (batch b = c//2, head-group hg = c%2, 8 heads each),
heads in 2 groups g of 4.

Key structure (vs v1): one global software pipeline so the ACT engine
(softmax exp, the roofline bottleneck at ~150us/core) never starves:
  - Latent projections (fused q2l/kv2l folded into c_attn on host) are
    emitted as sub-units dispensed between attention iterations.
  - Attention in [keys, queries] layout, 256-query blocks, 4 heads per
    exp instruction; scores row-group-tiled (4 concurrent K=32 matmuls),
    att@V col-group-tiled (4 concurrent M=32 matmuls).
  - Softmax denominator via an all-ones [128,32] stationary operand:
    z comes out replicated over the 32 latent partitions of each head, so
    normalization is reciprocal + one multiply straight out of PSUM.
  - 2-rank AllGather (pairs) of normalized latents, chunked per 512
    queries and overlapped; folded l2o+cproj ("W2") output projection
    per chunk, overlapped with later attention blocks.
"""
import sys

if "/opt/trn_rl_repo" not in sys.path:
    sys.path.insert(0, "/opt/trn_rl_repo")

import numpy as np
import ml_dtypes

import concourse.bass as bass
import concourse.tile as tile
from concourse.tile_rust import add_dep_helper
from concourse import bacc, mybir
from concourse.bass_utils import run_bass_kernel_spmd

F32 = mybir.dt.float32
F32R = mybir.dt.float32r
BF16 = mybir.dt.bfloat16

B, T, C = 4, 2048, 1024
H, HD, L = 16, 64, 32
NCORES = 8
QB = 256               # query block
NQB = T // QB          # 8
NB = T // 512          # 4 projection column blocks
KC = T // 128          # 16 key chunks of 128
SCALE = float(1.0 / np.sqrt(L))
REPLICA_GROUPS = [[0, 1], [2, 3], [4, 5], [6, 7]]

_CACHE = {}


def build_program(repeat=1):
    nc = bacc.Bacc("TRN2", target_bir_lowering=False, debug=False,
                   num_devices=NCORES)
    xT = nc.dram_tensor("xT", [C, T], F32R, kind="ExternalInput").ap()
    wlat = nc.dram_tensor("wlat", [C, 768], F32R, kind="ExternalInput").ap()
    qkbias = nc.dram_tensor("qkbias", [128, 4], F32, kind="ExternalInput").ap()
    p4 = nc.dram_tensor("p4", [128, 128], BF16, kind="ExternalInput").ap()
    ropec = nc.dram_tensor("ropec", [128, T], BF16, kind="ExternalInput").ap()
    ropes = nc.dram_tensor("ropes", [128, T], BF16, kind="ExternalInput").ap()
    wqk4 = nc.dram_tensor("wqk4", [128, 32], BF16, kind="ExternalInput").ap()
    wqkb4 = nc.dram_tensor("wqkb4", [128, 1], F32, kind="ExternalInput").ap()
    mask4 = nc.dram_tensor("mask4", [128, 512], BF16, kind="ExternalInput").ap()
    w2 = nc.dram_tensor("w2", [512, 512], BF16, kind="ExternalInput").ap()
    bout = nc.dram_tensor("bout", [128, 4], F32, kind="ExternalInput").ap()
    outT = nc.dram_tensor("outT", [512, T], F32, kind="ExternalOutput").ap()

    with tile.TileContext(nc) as tc:
        with tc.tile_pool(name="persist", bufs=1) as pp, \
             tc.tile_pool(name="work", bufs=1) as pw, \
             tc.tile_pool(name="ps", bufs=1, space="PSUM") as psp, \
             tc.tile_pool(name="dram", bufs=1, space="DRAM") as dr:
            pools = (pp, pw, psp, dr)
            for _rep in range(repeat):
                _emit_body(nc, tc, pools, xT, wlat, qkbias, p4, ropec, ropes,
                           wqk4, wqkb4, mask4, w2, bout, outT)
    nc.compile()
    return nc


def _emit_body(nc, tc, pools, xT, wlat, qkbias, p4, ropec, ropes, wqk4,
               wqkb4, mask4, w2, bout, outT):
    Iden = mybir.ActivationFunctionType.Identity
    Exp = mybir.ActivationFunctionType.Exp

    pp, pw, psp, dr = pools
    if True:
        wqk_t = pp.tile([128, 32], BF16, name="wqk_t", tag="wqk_t", bufs=2)
        nc.sync.dma_start(wqk_t[:], wqk4[:])
        wqkb_t = pp.tile([128, 1], F32, name="wqkb_t", tag="wqkb_t", bufs=2)
        nc.sync.dma_start(wqkb_t[:], wqkb4[:])
        qkb_t = pp.tile([128, 4], F32, name="qkb_t", tag="qkb_t", bufs=2)
        nc.sync.dma_start(qkb_t[:], qkbias[:])
        bout_t = pp.tile([128, 4], F32, name="bout_t", tag="bout_t", bufs=2)
        nc.sync.dma_start(bout_t[:], bout[:])
        mask_t = pp.tile([128, 512], BF16, name="mask_t", tag="mask_t", bufs=2)
        nc.sync.dma_start(mask_t[:], mask4[:])
        p4_t = pp.tile([128, 128], BF16, name="p4_t", tag="p4_t", bufs=2)
        nc.sync.dma_start(p4_t[:], p4[:])
        w2t = [pp.tile([128, 512], BF16, name=f"w2t{j}", tag=f"w2t{j}", bufs=2) for j in range(4)]
        for j in range(4):
            nc.sync.dma_start(w2t[j][:], w2[128 * j:128 * (j + 1), :])
        cos_t = pp.tile([128, T], BF16, name="cos_t", tag="cos_t", bufs=1)
        nc.sync.dma_start(cos_t[:], ropec[:])
        sin_t = pp.tile([128, T], BF16, name="sin_t", tag="sin_t", bufs=1)
        nc.sync.dma_start(sin_t[:], ropes[:])
        wl = pp.tile([128, 8 * 768], F32R, name="wl", tag="wl", bufs=1)
        for k in range(8):
            nc.sync.dma_start(wl[:, k * 768:(k + 1) * 768],
                              wlat[k * 128:(k + 1) * 128, :])
        ones_t = pp.tile([128, 32], BF16, name="ones_t", tag="ones_t", bufs=2)
        nc.vector.memset(ones_t[:], 1.0)

        # persistent activations; heads h at partitions 32*(h%2)..+32,
        # column half h//2 (so score waves serialize on array rows instead
        # of clashing on psum banks)
        QFc = pp.tile([128, 2 * T], BF16, name="QFc", tag="QFc", bufs=2)
        KRc = pp.tile([128, 2 * T], BF16, name="KRc", tag="KRc", bufs=2)
        QF = [QFc[0:64, :], QFc[64:128, :]]
        KR = [KRc[0:64, :], KRc[64:128, :]]
        VA = [pp.tile([128, KC * 128], BF16, name=f"VA{g}", tag=f"VA{g}", bufs=2) for g in range(2)]

        if True:
            ybounce = [dr.tile([256, 512], BF16, name=f"ybounce{p}", tag=f"yb{p}", bufs=2)
                       for p in range(NB)]
            ygath = [dr.tile([512, 512], BF16, name=f"ygath{p}", tag=f"yg{p}", bufs=2)
                     for p in range(NB)]
            ybounce_h = [dr.tile([256, 256], BF16, name=f"ybounce_h{i}", tag=f"ybh{i}", bufs=2)
                         for i in range(2)]
            ygath_h = [dr.tile([512, 256], BF16, name=f"ygath_h{i}", tag=f"ygh{i}", bufs=2)
                       for i in range(2)]

            xts = {}

            def load_xts(nb4):
                t = pw.tile([128, 8 * 512], F32R, name=f"xts{nb4}",
                            tag="xts", bufs=2)
                for k in range(8):
                    nc.sync.dma_start(
                        t[:, k * 512:(k + 1) * 512],
                        xT[k * 128:(k + 1) * 128, nb4 * 512:(nb4 + 1) * 512])
                xts[nb4] = t

            # ---------------- phase-A sub-unit emitters ---------------------
            def unit_lat(nb4, g, kind, evict_act):
                """x->latent projection + rope for 4 heads (one m-chunk)."""
                sl = slice(nb4 * 512, (nb4 + 1) * 512)
                mcol = (0 if kind == "q" else 2) + g
                lat_ps = psp.tile([128, 512], F32, name="lat_ps", tag="lat",
                                  bufs=2)
                for k in range(8):
                    nc.tensor.matmul(
                        lat_ps[:],
                        wl[:, k * 768 + mcol * 128: k * 768 + (mcol + 1) * 128],
                        xts[nb4][:, k * 512:(k + 1) * 512],
                        start=(k == 0), stop=(k == 7))
                a_sb = pw.tile([128, 512], BF16, name="a_sb", tag="a_sb",
                               bufs=3)
                if evict_act:
                    nc.scalar.activation(a_sb[:], lat_ps[:], Iden,
                                         bias=qkb_t[:, mcol:mcol + 1])
                else:
                    nc.vector.tensor_scalar_add(a_sb[:], lat_ps[:],
                                                qkb_t[:, mcol:mcol + 1])
                rot_ps = psp.tile([128, 512], F32, name="rot_ps", tag="lat",
                                  bufs=2)
                nc.tensor.matmul(rot_ps[:], p4_t[:], a_sb[:],
                                 start=True, stop=True)
                eng = nc.vector if _CACHE.get("no_gpsimd") else nc.gpsimd
                t1 = pw.tile([128, 512], BF16, name="t1", tag="t1", bufs=2)
                eng.tensor_mul(t1[:], a_sb[:], cos_t[:, sl])
                t2 = pw.tile([128, 512], BF16, name="t2", tag="t2", bufs=2)
                nc.vector.tensor_mul(t2[:], rot_ps[:], sin_t[:, sl])
                if kind == "k":
                    eng.tensor_add(KR[g][:, sl], t1[0:64, :], t2[0:64, :])
                    eng.tensor_add(KR[g][:, T + nb4 * 512:T + (nb4 + 1) * 512],
                                   t1[64:128, :], t2[64:128, :])
                else:
                    qro = pw.tile([128, 512], BF16, name="qro", tag=f"qro{g}",
                                  bufs=2)
                    eng.tensor_add(qro[:], t1[:], t2[:])
                    return qro

            def unit_wqk(nb4, g, qro, evict_act):
                sl = slice(nb4 * 512, (nb4 + 1) * 512)
                wq_ps = psp.tile([128, 512], F32, name="wq_ps", tag="lat",
                                 bufs=2)
                for h in range(4):
                    nc.tensor.matmul(wq_ps[32 * h:32 * h + 32, :],
                                     wqk_t[32 * h:32 * h + 32, :],
                                     qro[32 * h:32 * h + 32, :],
                                     start=True, stop=True,
                                     tile_position=(32 * h, 32 * h))
                sl2 = slice(T + nb4 * 512, T + (nb4 + 1) * 512)
                if evict_act:
                    nc.scalar.activation(QF[g][:, sl], wq_ps[0:64, :], Iden,
                                         bias=wqkb_t[0:64, 0:1])
                    nc.scalar.activation(QF[g][:, sl2], wq_ps[64:128, :], Iden,
                                         bias=wqkb_t[0:64, 0:1])
                else:
                    nc.vector.tensor_scalar_add(QF[g][:, sl], wq_ps[0:64, :],
                                                wqkb_t[0:64, 0:1])
                    nc.vector.tensor_scalar_add(QF[g][:, sl2], wq_ps[64:128, :],
                                                wqkb_t[0:64, 0:1])

            def unit_v(nb4, tck2):
                """V latents for token chunk tck = 4*nb4 + tck2 (both g)."""
                tck = 4 * nb4 + tck2
                v_ps = psp.tile([128, 512], F32, name="v_ps", tag="lat",
                                bufs=2)
                for k in range(8):
                    nc.tensor.matmul(
                        v_ps[:, 0:256],
                        xts[nb4][:, k * 512 + tck2 * 128: k * 512 + tck2 * 128 + 128],
                        wl[:, k * 768 + 512: k * 768 + 768],
                        start=(k == 0), stop=(k == 7))
                for g in range(2):
                    nc.vector.tensor_copy(
                        VA[g][:, tck * 128:(tck + 1) * 128],
                        v_ps[:, 128 * g:128 * (g + 1)])

            def emit_a_units(nb4, evict_act, gs=(0, 1), with_v=True):
                units = []
                for g in gs:
                    units.append(lambda g=g: unit_lat(nb4, g, "k", evict_act))

                    def qchain(g=g):
                        qro = unit_lat(nb4, g, "q", evict_act)
                        unit_wqk(nb4, g, qro, evict_act)
                    units.append(qchain)
                if with_v:
                    for tck2 in range(4):
                        units.append(lambda t=tck2: unit_v(nb4, t))
                return units

            # ---------------- phase-B unit -----------------------------------
            def emit_b(g, qb2, dispense):
                q0 = qb2 * QB
                nch = 2 * qb2 + 2
                so = _CACHE.get("scores_only")
                yz = None if so else psp.tile([128, 512], F32, name="yz",
                                              tag="acc", bufs=2)
                for kc in range(nch):
                    k0 = kc * 128
                    c0 = max(0, k0 - q0)
                    ncol = QB - c0
                    sp = psp.tile([128, 1024], F32, name="sp", tag="sp",
                                  bufs=2)
                    # region r = 2*(h%2) + h//2: h0->0, h1->2, h2->1, h3->3.
                    # waves (h0,h1) then (h2,h3): same array rows as wave 0,
                    # so they serialize on the PE instead of clashing on the
                    # bank; their regions share banks with wave 0, hence
                    # start=False + pending-zero from wave 0's start.
                    smm = {}
                    for h in ([] if _CACHE.get("no_scores") else range(4)):
                        row = 64 * g + 32 * (h % 2)
                        half = h // 2
                        r = 2 * (h % 2) + half
                        first = (half == 0)
                        lrow = 32 * (h % 2)
                        smm[h] = nc.tensor.matmul(
                            sp[:, QB * r + c0: QB * (r + 1)],
                            KR[g][lrow:lrow + 32,
                                  half * T + k0:half * T + k0 + 128],
                            QF[g][lrow:lrow + 32,
                                  half * T + q0 + c0:half * T + q0 + QB],
                            start=first, stop=first,
                            skip_group_check=not first,
                            tile_position=(row, 0))
                        if not first:
                            add_dep_helper(smm[h].ins, smm[h - 2].ins,
                                           sync=False,
                                           reason="psum bank zero-mark order")
                    es = pw.tile([128, 1024], BF16, name="es", tag="es",
                                 bufs=3)
                    esv = es.rearrange("p (h n) -> p h n", h=4)
                    spv = sp.rearrange("p (h n) -> p h n", h=4)
                    if not _CACHE.get("no_scores") and not _CACHE.get("no_exp"):
                        nc.scalar.activation(esv[:, :, c0:QB], spv[:, :, c0:QB],
                                             Exp, scale=SCALE)
                    elif _CACHE.get("no_exp"):
                        nc.vector.tensor_copy(esv[:, :, c0:QB], spv[:, :, c0:QB])
                    if k0 >= q0 and not _CACHE.get("no_scores") and not _CACHE.get("no_mask"):  # diagonal chunk: mask triangle
                        mv = mask_t.rearrange("p (h n) -> p h n", h=4)
                        nc.vector.tensor_mul(esv[:, :, c0:c0 + 128],
                                             esv[:, :, c0:c0 + 128], mv)
                    # y (cols 0:256) and z (cols 256:512) share each bank;
                    # one accumulation group per 32-partition range: opened by
                    # the first y matmul (zero-region mark covers z too),
                    # closed by the last z matmul.
                    ymm = {}
                    for h in ([] if _CACHE.get("no_vz") or so else range(4)):
                        r = 2 * (h % 2) + h // 2
                        ymm[h] = nc.tensor.matmul(
                            yz[32 * h:32 * h + 32, c0:QB],
                            VA[g][:, kc * 128 + 32 * h: kc * 128 + 32 * h + 32],
                            es[:, QB * r + c0:QB * (r + 1)],
                            start=(kc == 0), stop=False,
                            skip_group_check=True,
                            tile_position=(0, 32 * h))
                    for h in ([] if _CACHE.get("no_vz") or _CACHE.get("no_z") or so else range(4)):
                        r = 2 * (h % 2) + h // 2
                        zmm = nc.tensor.matmul(
                            yz[32 * h:32 * h + 32, 256 + c0:256 + QB],
                            ones_t[:],
                            es[:, QB * r + c0:QB * (r + 1)],
                            start=False, stop=(kc == nch - 1),
                            skip_group_check=True,
                            tile_position=(0, 32 * h))
                        if kc == 0:
                            add_dep_helper(zmm.ins, ymm[h].ins, sync=False,
                                           reason="psum bank zero-mark order")
                    dispense()
                if so:
                    return
                # normalize straight out of PSUM (z replicated per latent row)
                r32 = pw.tile([128, 256], F32, name="r32", tag="r32", bufs=2)
                nc.vector.reciprocal(r32[:], yz[:, 256:512])
                yn = pw.tile([128, 256], BF16, name="yn", tag="yn", bufs=2)
                nc.vector.tensor_mul(yn[:], yz[:, 0:256], r32[:])
                p = qb2 // 2
                if p == NB - 1 and _CACHE.get("tail_split", 0):
                    nc.sync.dma_start(
                        ybounce_h[qb2 % 2][128 * g:128 * (g + 1), :], yn[:])
                else:
                    nc.sync.dma_start(
                        ybounce[p][128 * g:128 * (g + 1),
                                   256 * (qb2 % 2):256 * (qb2 % 2) + 256],
                        yn[:])

            # ---------------- W2 output projection ---------------------------
            def emit_w2(p):
                ygr = []
                for j in range(4):
                    t = pw.tile([128, 512], BF16, name=f"ygr{p}_{j}",
                                tag="ygr", bufs=8)
                    nc.sync.dma_start(t[:], ygath[p][128 * j:128 * (j + 1), :])
                    ygr.append(t)
                for m in range(4):
                    ps_o = psp.tile([128, 512], F32, name="ps_o", tag="acc",
                                    bufs=2)
                    for j in range(4):
                        nc.tensor.matmul(ps_o[:],
                                         w2t[j][:, m * 128:(m + 1) * 128],
                                         ygr[j][:],
                                         start=(j == 0), stop=(j == 3))
                    o_sb = pw.tile([128, 512], F32, name="o_sb", tag="o_sb",
                                   bufs=2)
                    nc.vector.tensor_scalar_add(o_sb[:], ps_o[:],
                                                bout_t[:, m:m + 1])
                    nc.sync.dma_start(outT[m * 128:(m + 1) * 128,
                                           512 * p:512 * (p + 1)], o_sb[:])

            def emit_gather_h(i):
                if _CACHE.get("no_collective"):
                    nc.sync.dma_start(ygath_h[i][0:256, :], ybounce_h[i][:])
                    nc.sync.dma_start(ygath_h[i][256:512, :], ybounce_h[i][:])
                else:
                    nc.gpsimd.collective_compute(
                        "AllGather", mybir.AluOpType.bypass,
                        replica_groups=REPLICA_GROUPS,
                        ins=[ybounce_h[i].opt()], outs=[ygath_h[i].opt()])

            def emit_w2_h(i):
                qb2 = 2 * (NB - 1) + i
                ygr = []
                for j in range(4):
                    t = pw.tile([128, 256], BF16, name=f"ygrh{i}_{j}",
                                tag="ygrh", bufs=8)
                    nc.sync.dma_start(t[:], ygath_h[i][128 * j:128 * (j + 1), :])
                    ygr.append(t)
                for m in range(4):
                    ps_o = psp.tile([128, 512], F32, name="ps_o", tag="acc",
                                    bufs=2)
                    for j in range(4):
                        nc.tensor.matmul(ps_o[:, 0:256],
                                         w2t[j][:, m * 128:(m + 1) * 128],
                                         ygr[j][:],
                                         start=(j == 0), stop=(j == 3))
                    o_sb = pw.tile([128, 256], F32, name="o_sbh", tag="o_sbh",
                                   bufs=2)
                    nc.vector.tensor_scalar_add(o_sb[:], ps_o[:, 0:256],
                                                bout_t[:, m:m + 1])
                    nc.sync.dma_start(outT[m * 128:(m + 1) * 128,
                                           256 * qb2:256 * (qb2 + 1)], o_sb[:])

            def emit_gather(p):
                if _CACHE.get("no_collective"):
                    nc.sync.dma_start(ygath[p][0:256, :], ybounce[p][:])
                    nc.sync.dma_start(ygath[p][256:512, :], ybounce[p][:])
                else:
                    nc.gpsimd.collective_compute(
                        "AllGather", mybir.AluOpType.bypass,
                        replica_groups=REPLICA_GROUPS,
                        ins=[ybounce[p].opt()], outs=[ygath[p].opt()])

            # ---------------- top-level schedule -----------------------------
            load_xts(0)
            # head of A(0): everything B(g0, qb 0/1) needs, evictions on ACT
            for u in emit_a_units(0, evict_act=True, gs=(0,), with_v=False):
                u()
            for tck2 in range(4):
                unit_v(0, tck2)

            if _CACHE.get("a_only"):
                for u in emit_a_units(0, evict_act=True, gs=(1,), with_v=False):
                    u()
                for nb4 in range(1, 4):
                    load_xts(nb4)
                    for u in emit_a_units(nb4, evict_act=True):
                        u()
                return
            # dispensers per block: remaining A units, paced over B iters
            pending = {
                0: emit_a_units(0, evict_act=False, gs=(1,), with_v=False)
                + emit_a_units(1, evict_act=False),
                1: emit_a_units(2, evict_act=False),
                2: emit_a_units(3, evict_act=False),
                3: [],
            }
            if _CACHE.get("no_dispense"):
                for pbl in range(4):
                    if pbl < 3:
                        load_xts(pbl + 1)
                    for u in pending[pbl]:
                        u()
                    pending[pbl] = []
            iters_in_block = {p: 2 * ((4 * p + 2) + (4 * p + 4))
                              for p in range(NB)}

            for p in range(NB):
                if p < 3:
                    load_xts(p + 1)
                units = pending[p]
                n_iter = iters_in_block[p]
                state = {"i": 0, "done": 0}

                def dispense(units=units, n_iter=n_iter, state=state):
                    state["i"] += 1
                    target = (state["i"] * len(units) + n_iter - 1) // n_iter
                    while state["done"] < min(target, len(units)):
                        units[state["done"]]()
                        state["done"] += 1

                last = (p == NB - 1) and _CACHE.get("tail_split", 0) \
                    and not _CACHE.get("no_w2")
                if last:
                    unit_order = [(0, 2 * p), (1, 2 * p),
                                  (0, 2 * p + 1), (1, 2 * p + 1)]
                else:
                    unit_order = [(0, 2 * p), (0, 2 * p + 1),
                                  (1, 2 * p), (1, 2 * p + 1)]
                bcount = 0
                for g, qb2 in unit_order:
                    if bcount == 2 and p >= 1 and not _CACHE.get("no_w2"):
                        emit_w2(p - 1)
                    emit_b(g, qb2, dispense)
                    bcount += 1
                    if last and bcount == 2:
                        emit_gather_h(0)
                    if last and bcount == 3:
                        emit_w2_h(0)
                while state["done"] < len(units):
                    units[state["done"]]()
                    state["done"] += 1
                if not _CACHE.get("no_w2"):
                    if last:
                        emit_gather_h(1)
                        emit_w2_h(1)
                    else:
                        emit_gather(p)
            if _CACHE.get("no_w2"):
                pass
            elif not _CACHE.get("tail_split", 0):
                emit_w2(NB - 1)


# ---------------------------------------------------------------------------
# Host-side input preparation
# ---------------------------------------------------------------------------

def prepare_inputs(inputs):
    """Fold weights and build the 8 per-core input maps."""
    x = np.ascontiguousarray(np.asarray(inputs["x"], dtype=np.float32))
    caw = np.asarray(inputs["c_attn_w"], dtype=np.float32)
    cab = np.asarray(inputs["c_attn_b"], dtype=np.float32)
    q2l = np.asarray(inputs["q2l_w"], dtype=np.float32)
    q2lb = np.asarray(inputs["q2l_b"], dtype=np.float32)
    kv2l = np.asarray(inputs["kv2l_w"], dtype=np.float32)
    kv2lb = np.asarray(inputs["kv2l_b"], dtype=np.float32)
    l2o = np.asarray(inputs["l2o_w"], dtype=np.float32)
    l2ob = np.asarray(inputs["l2o_b"], dtype=np.float32)
    wqk = np.asarray(inputs["wqk_w"], dtype=np.float32)
    wqkb = np.asarray(inputs["wqk_b"], dtype=np.float32)
    cpw = np.asarray(inputs["cproj_w"], dtype=np.float32)
    cpb = np.asarray(inputs["cproj_b"], dtype=np.float32)

    # rope tables [L, T] -> tiled to [128, T]
    inv_freq = 1.0 / (10000.0 ** (np.arange(0, L, 2, dtype=np.float32) / L))
    t_ar = np.arange(T, dtype=np.float32)
    freqs = np.outer(t_ar, inv_freq)
    cosT = np.repeat(np.cos(freqs), 2, axis=-1)[:, :L].T.astype(np.float32)
    sinT = np.repeat(np.sin(freqs), 2, axis=-1)[:, :L].T.astype(np.float32)
    ropec = np.tile(cosT, (4, 1)).astype(ml_dtypes.bfloat16)   # [128, T]
    ropes = np.tile(sinT, (4, 1)).astype(ml_dtypes.bfloat16)

    P = np.zeros((L, L), np.float32)
    for i in range(L // 2):
        P[2 * i, 2 * i + 1] = -1.0
        P[2 * i + 1, 2 * i] = 1.0

    def fold_head(h):
        Wq = caw[h * HD:(h + 1) * HD, :]
        Wk = caw[C + h * HD: C + (h + 1) * HD, :]
        Wv = caw[2 * C + h * HD: 2 * C + (h + 1) * HD, :]
        bq = cab[h * HD:(h + 1) * HD]
        bk = cab[C + h * HD: C + (h + 1) * HD]
        bv = cab[2 * C + h * HD: 2 * C + (h + 1) * HD]
        return (q2l @ Wq, kv2l @ Wk, kv2l @ Wv,
                q2l @ bq + q2lb, kv2l @ bk + kv2lb, kv2l @ bv + kv2lb)

    # W2 [H*L, C] + folded output bias
    W2 = np.zeros((H * L, C), np.float32)
    b_out = cpb.astype(np.float64).copy()
    for h in range(H):
        W2_h = l2o.T @ cpw[:, h * HD:(h + 1) * HD].T
        W2[h * L:(h + 1) * L] = W2_h
        _, _, _, _, _, bvl = fold_head(h)
        b_out += bvl @ W2_h
        b_out += l2ob @ cpw[:, h * HD:(h + 1) * HD].T
    b_out = b_out.astype(np.float32)

    # per-head-group folded projection stacks
    wlat_hg, qkb_hg, bout_hg = [], [], []
    for hg in range(2):
        wlat = np.zeros((C, 768), np.float32)
        qkb = np.zeros((128, 4), np.float32)
        for g in range(2):
            for lh4 in range(4):
                lh = 4 * g + lh4
                h = hg * 8 + lh
                Wql, Wkl, Wvl, bql, bkl, bvl = fold_head(h)
                wlat[:, (0 + g) * 128 + lh4 * 32:(0 + g) * 128 + lh4 * 32 + 32] = Wql.T
                wlat[:, (2 + g) * 128 + lh4 * 32:(2 + g) * 128 + lh4 * 32 + 32] = Wkl.T
                qkb[lh4 * 32:lh4 * 32 + 32, 0 + g] = bql
                qkb[lh4 * 32:lh4 * 32 + 32, 2 + g] = bkl
        for lh in range(8):
            h = hg * 8 + lh
            _, _, Wvl, _, _, _ = fold_head(h)
            wlat[:, 512 + lh * 32: 512 + (lh + 1) * 32] = Wvl.T
        wlat_hg.append(wlat)
        qkb_hg.append(qkb)
        bo = b_out[hg * 512:(hg + 1) * 512]
        bout_hg.append(np.ascontiguousarray(bo.reshape(4, 128).T))

    # w2 per hg: gathered row r = 256*rank + 128*g + 32*h + l == W2 row
    w2_hg = [np.ascontiguousarray(W2[:, hg * 512:(hg + 1) * 512])
             .astype(ml_dtypes.bfloat16) for hg in range(2)]

    wqk4 = np.tile(wqk.T, (4, 1)).astype(ml_dtypes.bfloat16)     # [128, 32]
    wqkb4 = np.tile(wqkb, 4).reshape(128, 1).astype(np.float32)

    i_idx = np.arange(128)[:, None]
    u_idx = np.arange(128)[None, :]
    tri = (u_idx >= i_idx).astype(ml_dtypes.bfloat16)            # [128, 128]
    mask4 = np.tile(tri, (1, 4))                                 # [128, 512]

    # block-diag rotation lhsT: out = p4.T @ lat = P @ lat per 32-block
    p4 = np.zeros((128, 128), np.float32)
    for h in range(4):
        p4[h * 32:(h + 1) * 32, h * 32:(h + 1) * 32] = P.T
    p4 = p4.astype(ml_dtypes.bfloat16)

    xT_b = [np.ascontiguousarray(x[b].T) for b in range(B)]

    in_maps = []
    for core in range(NCORES):
        b, hg = core // 2, core % 2
        in_maps.append({
            "xT": xT_b[b],
            "wlat": wlat_hg[hg],
            "qkbias": qkb_hg[hg],
            "ropec": ropec,
            "ropes": ropes,
            "wqk4": wqk4,
            "wqkb4": wqkb4,
            "mask4": mask4,
            "p4": p4,
            "w2": w2_hg[hg],
            "bout": bout_hg[hg],
        })
    return in_maps


def assemble_output(results):
    out = np.zeros((B, T, C), np.float32)
    for core in range(NCORES):
        b, hg = core // 2, core % 2
        out[b, :, hg * 512:(hg + 1) * 512] = results[core]["outT"].T
    return out


def kernel(**inputs):
    if "nc" not in _CACHE:
        _CACHE["nc"] = build_program()
    nc = _CACHE["nc"]
    in_maps = prepare_inputs(inputs)
    # The neuron runtime is occasionally left unrecoverable by a previous
    # process (NRT_EXEC_UNIT_UNRECOVERABLE); a short wait + retry clears it.
    last = None
    for attempt in range(3):
        try:
            res = run_bass_kernel_spmd(nc, in_maps,
                                       core_ids=list(range(NCORES)))
            return assemble_output(res.results)
        except Exception as e:  # noqa: BLE001
            last = e
            import time as _time
            _time.sleep(10 * (attempt + 1))
    raise last


# ---------------------------------------------------------------------------
# Timing runner (dev/test only): keeps the compiled executable and
# device-staged inputs so repeated executions measure device time + dispatch,
# not host transfers or recompiles.
# ---------------------------------------------------------------------------

class Runner:
    def __init__(self, nc, in_maps):
        import jax
        from jax.sharding import Mesh, PartitionSpec, NamedSharding
        from jax.experimental.shard_map import shard_map
        from concourse import bass2jax, mybir as _mybir

        bass2jax.install_neuronx_cc_hook()
        partition_name = (nc.partition_id_tensor.name
                          if nc.partition_id_tensor else None)
        in_names, out_names, out_avals, zero_outs = [], [], [], []
        for alloc in nc.m.functions[0].allocations:
            if not isinstance(alloc, _mybir.MemoryLocationSet):
                continue
            name = alloc.memorylocations[0].name
            if alloc.kind == "ExternalInput":
                if name != partition_name:
                    in_names.append(name)
            elif alloc.kind == "ExternalOutput":
                shape = tuple(alloc.tensor_shape)
                dtype = _mybir.dt.np(alloc.dtype)
                out_names.append(name)
                out_avals.append(jax.core.ShapedArray(shape, dtype))
                zero_outs.append(np.zeros(shape, dtype))
        n_params = len(in_names)
        all_names = list(in_names) + list(out_names)
        if partition_name is not None:
            all_names.append(partition_name)
        self.out_names = out_names

        def _body(*args):
            operands = list(args)
            if partition_name is not None:
                operands.append(bass2jax.partition_id_tensor())
            outs = bass2jax._bass_exec_p.bind(
                *operands,
                out_avals=tuple(out_avals),
                in_names=tuple(all_names),
                out_names=tuple(out_names),
                lowering_input_output_aliases=(),
                sim_require_finite=True,
                sim_require_nnan=True,
                nc=nc,
            )
            return tuple(outs)

        devices = jax.devices()[:NCORES]
        mesh = Mesh(np.asarray(devices), ("core",))
        n_out = len(out_names)
        self._fn = jax.jit(shard_map(
            _body, mesh=mesh,
            in_specs=(PartitionSpec("core"),) * (n_params + n_out),
            out_specs=(PartitionSpec("core"),) * n_out,
            check_rep=False))
        sh = NamedSharding(mesh, PartitionSpec("core"))
        concat_in = [
            np.concatenate([np.asarray(in_maps[c][nm]) for c in range(NCORES)],
                           axis=0)
            for nm in in_names]
        concat_zeros = [np.zeros((NCORES * z.shape[0], *z.shape[1:]), z.dtype)
                        for z in zero_outs]
        self._staged = [jax.device_put(a, sh) for a in concat_in + concat_zeros]
        self._out_shapes = [a.shape for a in zero_outs]

    def run(self):
        import jax
        outs = self._fn(*self._staged)
        jax.block_until_ready(outs)
        return outs

    def results(self):
        outs = self.run()
        res = []
        for c in range(NCORES):
            d = {}
            for i, nm in enumerate(self.out_names):
                s0 = self._out_shapes[i][0]
                d[nm] = np.asarray(outs[i]).reshape(NCORES, s0, -1)[c]
            res.append(d)
        return res


if __name__ == "__main__":
    data = dict(np.load("/root/problem/inputs.npz"))
    expected = np.load("/root/problem/expected.npy")
    got = kernel(**data)
    err = np.abs(got - expected)
    print(f"absmax={err.max():.3e} rel={err.max() / np.abs(expected).max():.3e}")
